# revision 8
# baseline (speedup 1.0000x reference)
"""Tensor-parallel attention kernel for Trainium2 (8 NeuronCores).

Problem: B=2, L=2048, DIM=1024, H=16 heads, HD=64 (QKV proj + RoPE + SDPA + out proj).

Sharding: tensor-parallel over heads — 2 heads per core. Each core:
  - computes q/k/v for its 2 heads feature-major (qT/kT [128, 4096] = [2*64 hd, B*L]),
    via f32r matmuls of w_qkv column-slices against xT,
  - applies RoPE in-place (partition-swap via SBUF-SBUF DMA + DVE mul/add),
  - flash-style attention per (batch, 512-query tile): S^T chunks [128 keys, 512 q]
    on PE (2 heads row-packed), exp on ACT (psum -> f32r SBUF, fused 1/sqrt(hd) scale),
    P^T @ V accumulated on PE with a ones-column appended to V for the softmax
    denominator (M=65), normalization via DVE with DMA-broadcast reciprocal,
  - output projection against its w_out row-slice -> partial [4096, 1024].
Host sums the 8 partials (the "all-reduce after out_proj").
"""
import numpy as np

import concourse.bass as bass
import concourse.tile as tile
from concourse import bacc, mybir

B, L, DIM, H, HD = 2, 2048, 1024, 16, 64
NCORES = 8
HPC = H // NCORES            # heads per core = 2
T = B * L                    # 4096 tokens
NT = T // 512                # 8 token tiles of 512
KC = DIM // 128              # 8 contraction chunks for qkv
CH = T // 128                # 32 key chunks of 128 (global)
CHB = L // 128               # 16 key chunks per batch
QT = L // 512                # 4 query tiles per batch
VW = 2 * HD + 2              # v-nat chunk width: [V_A(64) | ones | V_B(64) | ones] = 130

F32 = mybir.dt.float32
F32R = mybir.dt.float32r
AF = mybir.ActivationFunctionType

_CACHE = {}


def _build_nc():
    nc = bacc.Bacc("TRN2", target_bir_lowering=False, debug=False)

    xT_d = nc.dram_tensor("xT", [DIM, T], F32R, kind="ExternalInput")
    wqkv_d = nc.dram_tensor("wqkv", [128, KC, 3, 128], F32R, kind="ExternalInput")
    wout_d = nc.dram_tensor("wout", [128, 2, 512], F32R, kind="ExternalInput")
    cos_d = nc.dram_tensor("cosr", [64, L], F32, kind="ExternalInput")
    c2_d = nc.dram_tensor("c2r", [64, L], F32, kind="ExternalInput")
    out_d = nc.dram_tensor("out", [T, DIM], F32, kind="ExternalOutput")

    def rep2(dram_ap):
        # DRAM [64, L] read twice -> stream of 128 rows (partition-doubling)
        return bass.AP(tensor=dram_ap.tensor, offset=dram_ap.offset,
                       ap=[[0, 2]] + list(dram_ap.ap))

    with tile.TileContext(nc) as tc:
        from contextlib import ExitStack
        with ExitStack() as ctx:
            const = ctx.enter_context(tc.tile_pool(name="const", bufs=1))
            big = ctx.enter_context(tc.tile_pool(name="big", bufs=1))

            wqkv_sb = const.tile([128, KC, 3, 128], F32R)
            wout_sb = const.tile([128, 2, 512], F32R)
            cos_sb = const.tile([128, T], F32)
            c2_sb = const.tile([128, T], F32)
            ident = const.tile([128, 128], F32)

            nc.sync.dma_start(out=wqkv_sb[:], in_=wqkv_d[:])
            nc.sync.dma_start(out=wout_sb[:], in_=wout_d[:])
            for b in range(B):
                sl = slice(b * L, (b + 1) * L)
                nc.sync.dma_start(out=cos_sb[:, sl], in_=rep2(cos_d[:]))
                nc.sync.dma_start(out=c2_sb[:, sl], in_=rep2(c2_d[:]))
            from concourse.masks import make_identity
            make_identity(nc, ident[:])

            qT = big.tile([128, T], F32R)
            kT = big.tile([128, T], F32R)
            vT = big.tile([128, T], F32)
            vnat = big.tile([128, CH, VW], F32R)
            OT = big.tile([128, T], F32R)

            ones_sb = const.tile([128, CH], F32)
            nc.vector.memset(ones_sb[:], 1.0)
            nc.vector.tensor_copy(vnat[:, :, 64], ones_sb[:])
            nc.vector.tensor_copy(vnat[:, :, 129], ones_sb[:])

            # ---------------- Phase 1: QKV projection + RoPE + vT ----------------
            with tc.tile_pool(name="qkv_ps", bufs=2, space="PSUM") as qkv_ps, \
                 tc.tile_pool(name="tr_ps", bufs=2, space="PSUM") as tr_ps, \
                 tc.tile_pool(name="xt_pool", bufs=3) as xt_pool, \
                 tc.tile_pool(name="rope_pool", bufs=4) as rope_pool:

                for nt in range(NT):
                    csl = slice(nt * 512, (nt + 1) * 512)
                    psq = qkv_ps.tile([128, 512], F32, tag="psq")
                    psk = qkv_ps.tile([128, 512], F32, tag="psk")
                    psv = qkv_ps.tile([128, 512], F32, tag="psv")
                    for kc in range(KC):
                        xt = xt_pool.tile([128, 512], F32R, tag="xt")
                        nc.sync.dma_start(
                            out=xt[:], in_=xT_d[kc * 128:(kc + 1) * 128, csl])
                        nc.tensor.matmul(psq[:], wqkv_sb[:, kc, 0, :], xt[:],
                                         start=(kc == 0), stop=(kc == KC - 1))
                        nc.tensor.matmul(psk[:], wqkv_sb[:, kc, 1, :], xt[:],
                                         start=(kc == 0), stop=(kc == KC - 1))
                        nc.tensor.matmul(psv[:], wqkv_sb[:, kc, 2, :], xt[:],
                                         start=(kc == 0), stop=(kc == KC - 1))

                    # RoPE: dst = ps*cos + shift64(ps*c2); shift swaps 32-rows
                    # within each 64-block (c2 has signs+shift pre-folded).
                    for ps, dst in ((psq, qT), (psk, kT)):
                        u = rope_pool.tile([128, 512], F32, tag="u")
                        m1 = rope_pool.tile([128, 512], F32, tag="m1")
                        tmp = rope_pool.tile([128, 512], F32, tag="tmp")
                        nc.vector.tensor_mul(u[:], ps[:], c2_sb[:, csl])
                        for blk in range(4):
                            src = u[(blk ^ 1) * 32:(blk ^ 1) * 32 + 32, :]
                            nc.sync.dma_start(
                                out=tmp[blk * 32:(blk + 1) * 32, :], in_=src)
                        nc.vector.tensor_mul(m1[:], ps[:], cos_sb[:, csl])
                        nc.vector.tensor_add(dst[:, csl], m1[:], tmp[:])

                    nc.vector.tensor_copy(vT[:, csl], psv[:])

                    # V transpose for this token tile: 4 chunks of 128 keys
                    for j in range(4):
                        c = nt * 4 + j
                        pst = tr_ps.tile([128, 128], F32, tag="pst")
                        nc.tensor.transpose(
                            pst[:], vT[:, c * 128:(c + 1) * 128], ident[:])
                        nc.vector.tensor_copy(vnat[:, c, 0:64], pst[:, 0:64])
                        nc.vector.tensor_copy(vnat[:, c, 65:129], pst[:, 64:128])

            # ---------------- Phase 2: attention + out projection ----------------
            with tc.tile_pool(name="s_ps", bufs=3, space="PSUM") as s_ps, \
                 tc.tile_pool(name="o_ps", bufs=1, space="PSUM") as o_ps, \
                 tc.tile_pool(name="op_ps", bufs=2, space="PSUM") as op_ps, \
                 tc.tile_pool(name="pt_pool", bufs=4) as pt_pool, \
                 tc.tile_pool(name="d_pool", bufs=4) as d_pool, \
                 tc.tile_pool(name="o_pool", bufs=4) as o_pool, \
                 tc.tile_pool(name="st_pool", bufs=3) as st_pool:

                for b in range(B):
                    for qt in range(QT):
                        qsl = slice(b * L + qt * 512, b * L + qt * 512 + 512)
                        oA = o_ps.tile([65, 512], F32, tag="oA")
                        oB = o_ps.tile([65, 512], F32, tag="oB")
                        for c in range(CHB):
                            cg = b * CHB + c
                            ksl = slice(cg * 128, (cg + 1) * 128)
                            first, last = (c == 0), (c == CHB - 1)
                            for hi, o_ps_tile in ((0, oA), (1, oB)):
                                hsl = slice(hi * 64, hi * 64 + 64)
                                s = s_ps.tile([128, 512], F32, tag="s")
                                nc.tensor.matmul(
                                    s[:], kT[hsl, ksl], qT[hsl, qsl])
                                pt = pt_pool.tile([128, 512], F32R, tag="pt")
                                nc.scalar.activation(
                                    pt[:], s[:], AF.Exp, scale=float(HD ** -0.5))
                                nc.tensor.matmul(
                                    o_ps_tile[:],
                                    vnat[:, cg, hi * 65:(hi + 1) * 65], pt[:],
                                    start=first, stop=last)

                        # normalize: OT[h] = o[h] * (1/d[h]) broadcast over 64 rows
                        for hi, o_ps_tile in ((0, oA), (1, oB)):
                            dT = d_pool.tile([65, 512], F32, tag="dT")
                            rd = d_pool.tile([64, 512], F32, tag="rd")
                            nc.vector.reciprocal(dT[64:65, :], o_ps_tile[64:65, :])
                            nc.sync.dma_start(out=dT[0:1, :], in_=dT[64:65, :])
                            nc.gpsimd.partition_broadcast(
                                rd[:], dT[0:1, :], channels=64)
                            if hi == 0:
                                nc.vector.tensor_mul(
                                    OT[0:64, qsl], o_ps_tile[0:64, :], rd[:])
                            else:
                                otb = o_pool.tile([64, 512], F32R, tag="otb")
                                nc.vector.tensor_mul(
                                    otb[:], o_ps_tile[0:64, :], rd[:])
                                nc.sync.dma_start(out=OT[64:128, qsl], in_=otb[:])

                        # out projection for the 4 token chunks of this q tile
                        for j in range(4):
                            tch = (b * L + qt * 512) // 128 + j
                            tsl = slice(tch * 128, (tch + 1) * 128)
                            for dj in range(2):
                                po = op_ps.tile([128, 512], F32, tag="po")
                                nc.tensor.matmul(
                                    po[:], OT[:, tsl], wout_sb[:, dj, :])
                                st = st_pool.tile([128, 512], F32, tag="st")
                                nc.vector.tensor_copy(st[:], po[:])
                                nc.sync.dma_start(
                                    out=out_d[tsl, dj * 512:(dj + 1) * 512],
                                    in_=st[:])

    nc.compile()
    return nc


def _host_prep(x, cos, sin, w_qkv, w_out):
    x = np.asarray(x, dtype=np.float32)
    cos = np.asarray(cos, dtype=np.float32)
    sin = np.asarray(sin, dtype=np.float32)
    w_qkv = np.asarray(w_qkv, dtype=np.float32)
    w_out = np.asarray(w_out, dtype=np.float32)

    xT = np.ascontiguousarray(x.reshape(T, DIM).T)           # [DIM, T]
    cosr = np.ascontiguousarray(cos.T)                       # [64, L]
    sinT = sin.T                                             # [64, L]
    c2 = np.empty_like(sinT)
    c2[0:32] = sinT[32:64]        # u[j] (j<32) carries +sin[j+32] -> shifts to p=j+32
    c2[32:64] = -sinT[0:32]       # u[j] (j>=32) carries -sin[j-32] -> shifts to p=j-32
    c2 = np.ascontiguousarray(c2)

    in_maps = []
    for c in range(NCORES):
        h0 = c * HPC
        fs = slice(h0 * HD, h0 * HD + HPC * HD)              # 128 feature cols
        wc = np.concatenate(
            [w_qkv[:, 0 * H * HD:][:, fs],
             w_qkv[:, 1 * H * HD:][:, fs],
             w_qkv[:, 2 * H * HD:][:, fs]], axis=1)          # [1024, 384] = q|k|v
        # [kc*128+p, m*128+f] -> [p, kc, m, f]
        wq = np.ascontiguousarray(
            wc.reshape(KC, 128, 3, 128).transpose(1, 0, 2, 3))
        wo = np.ascontiguousarray(
            w_out[fs, :].reshape(128, 2, 512))               # [128, 2, 512]
        in_maps.append({
            "xT": xT, "wqkv": wq, "wout": wo, "cosr": cosr, "c2r": c2,
        })
    return in_maps


def _get_runner():
    if "runner" in _CACHE:
        return _CACHE["runner"]

    import jax
    from jax.sharding import Mesh, PartitionSpec
    from jax.experimental.shard_map import shard_map
    from concourse import bass2jax

    nc = _build_nc()
    bass2jax.install_neuronx_cc_hook()

    in_names = ["xT", "wqkv", "wout", "cosr", "c2r"]
    out_names = ["out"]
    out_avals = [jax.core.ShapedArray((T, DIM), np.float32)]
    bind_names = in_names + out_names
    if nc.partition_id_tensor is not None:
        bind_names = bind_names + [nc.partition_id_tensor.name]

    def _body(*args):
        operands = list(args)
        if nc.partition_id_tensor is not None:
            operands.append(bass2jax.partition_id_tensor())
        outs = bass2jax._bass_exec_p.bind(
            *operands,
            out_avals=tuple(out_avals),
            in_names=tuple(bind_names),
            out_names=tuple(out_names),
            lowering_input_output_aliases=(),
            sim_require_finite=True,
            sim_require_nnan=True,
            nc=nc,
        )
        return tuple(outs)

    devices = jax.devices()[:NCORES]
    mesh = Mesh(np.asarray(devices), ("core",))
    in_specs = (PartitionSpec("core"),) * (len(in_names) + 1)
    out_specs = (PartitionSpec("core"),)
    sharded = jax.jit(
        shard_map(_body, mesh=mesh, in_specs=in_specs, out_specs=out_specs,
                  check_rep=False),
        donate_argnums=(len(in_names),),
        keep_unused=True,
    )
    _CACHE["runner"] = (sharded, in_names)
    return _CACHE["runner"]


def device_inputs(in_maps):
    """Concatenate per-core input maps along axis 0 in runner arg order."""
    _, in_names = _get_runner()
    return [
        np.concatenate([np.asarray(m[name]) for m in in_maps], axis=0)
        for name in in_names
    ]


def run_sharded(in_maps):
    """Run the SPMD kernel; returns list of per-core output arrays [T, DIM]."""
    sharded, _ = _get_runner()
    concat_in = device_inputs(in_maps)
    zeros = np.zeros((NCORES * T, DIM), np.float32)
    (out,) = sharded(*concat_in, zeros)
    out = np.asarray(out).reshape(NCORES, T, DIM)
    return [out[c] for c in range(NCORES)]


def kernel(x, cos, sin, w_qkv, w_out):
    in_maps = _host_prep(x, cos, sin, w_qkv, w_out)
    parts = run_sharded(in_maps)
    full = parts[0].copy()
    for p in parts[1:]:
        full += p
    return full.reshape(B, L, DIM)


if __name__ == "__main__":
    rng = np.random.default_rng(0)
    x = rng.standard_normal((B, L, DIM), dtype=np.float32)
    import reference
    inputs = reference.setup_inputs()
    out = kernel(**{k: np.asarray(v) for k, v in inputs.items()})
    ref = np.asarray(reference.reference(**inputs))
    err = np.abs(out - ref)
    rel = np.sqrt((err ** 2).mean()) / np.sqrt((ref ** 2).mean())
    print("rms rel:", rel, "max abs:", err.max())
